# revision 1
# baseline (speedup 1.0000x reference)
"""CARAFE D2: tensor-engine banded-matmul design.

out[c, y, x] = sum_di sum_dj fpad[c, y//2+di, x//2+dj] * m[di*5+dj, y, x]

For a fixed input row index i (covering output rows y=2i and 2i+1, which use
the same feature rows) and tap row di, the contribution over all (yp, x) is a
matmul contracting over the padded input column j' (128 lanes):

    out_i[c, (yp, x)] += sum_{j'} ftT[j', r=i+di, c] * B_di[j', (yp, x)]

where B_di[j', yp, x] = m[(di, dj), 2i+yp, x] at dj = j' - x//2 + 2 (banded,
5 diagonals per yp, zeros elsewhere).  The 5 di-taps accumulate in a full
512-wide PSUM bank.  B tiles are built per i by GPSIMD local_scatter from a
host-pregathered dense tensor maskD with a static index table (fp32 values
scattered as uint16 pairs; the banded slot positions are y-independent).

The local_scatter extended-ISA instruction cannot carry semaphore ops through
this walrus build, so its sync is relocated onto adjacent Pool-engine memsets
(sound: Q7 execution is strict FIFO per engine), and a final pass splits any
instruction with more than one wait into standalone sequencer NOPs.
"""

import os

import numpy as np

import concourse.bass as bass
import concourse.mybir as mybir
import concourse.tile as tile
from concourse import library_config

F32 = mybir.dt.float32
U16 = mybir.dt.uint16
I16 = mybir.dt.int16
_add_dep = bass._add_dep_helper

N, C, H, W = 2, 256, 128, 128
K = 5
S = 2
PAD = K // 2
SH, SW = H * S, W * S

N_CORES = 8
QH = H // 4          # 32 input rows per core
R_IN = QH + 2 * PAD  # 36 padded feature rows per core
N_I = QH             # 32 output row-pairs per core
YB = 8               # y rows per output DMA batch (4 i's)
RCH = 4              # feature rows per load chunk
NSL3 = 3 * K * 2 * 2  # uint16 scatter slots for the di 0..2 triple
NSL2 = 2 * K * 2 * 2  # uint16 scatter slots for the di 3..4 pair
NSL = NSL3 + NSL2
BTP = K * SW + 2      # bt yp-row length: K*SW fp32 payload + 2 fp32 pad


def _mi(x):
    return getattr(x, "ins", x)


def relocate_sync(pres, scats, posts):
    """Move the scatters' semaphore waits onto `pres` and updates onto
    `posts` (all chained in Pool-engine program order via nosync deps; Q7
    execution is strict FIFO per engine, so advancing waits and delaying
    updates across the group is sync-preserving).  Waits merge by max per
    semaphore, updates merge by sum."""
    def si_of(inst):
        si = inst.sync_info
        if si is None:
            return [], []
        return list(si.on_wait or []), list(si.on_update or [])

    wmax, uacc = {}, {}
    for s in scats:
        w, u = si_of(_mi(s))
        for x in w:
            assert x.sync_type == "semaphore" and x.wait_mode == "sem-ge-imm", x
            prev = wmax.get(x.id)
            if prev is None or x.wait_value > prev.wait_value:
                wmax[x.id] = x
        for x in u:
            assert x.sync_type == "semaphore" and x.update_mode in (
                "sem-inc", "sem-add-imm"), x
            prev = uacc.get(x.id)
            if prev is None:
                uacc[x.id] = mybir.SyncUpdate(
                    sync_type="semaphore", id=x.id, ant_name=x.ant_name,
                    update_mode="sem-add-imm", update_value=x.update_value)
            else:
                prev.update_value = prev.update_value + x.update_value
        _mi(s).sync_info = mybir.SyncInfo(on_wait=[], on_update=[])

    for carrier in pres:
        ci = _mi(carrier)
        cw, cu = si_of(ci)
        for w in cw:
            inc = wmax.pop(w.id, None)
            if inc is not None and inc.wait_value > w.wait_value:
                w.wait_value = inc.wait_value
        take = list(wmax.values())
        wmax.clear()
        ci.sync_info = mybir.SyncInfo(on_wait=cw + take, on_update=cu)
        break
    assert not wmax

    for carrier in posts:
        ci = _mi(carrier)
        cw, cu = si_of(ci)
        for u in cu:
            inc = uacc.pop(u.id, None)
            if inc is not None:
                u.update_value = u.update_value + inc.update_value
                u.update_mode = "sem-add-imm"
        take = list(uacc.values())
        uacc.clear()
        ci.sync_info = mybir.SyncInfo(on_wait=cw, on_update=cu + take)
        break
    assert not uacc


def split_sync(nc):
    """Enforce <=1 wait and <=1 update per instruction (this walrus build's
    events capacity), hoisting excess waits onto standalone same-engine
    sequencer NOPs placed immediately before (sync-equivalent).  Also hoists
    a wait that shares its semaphore with the instruction's own update."""
    for f in nc.m.functions:
        for b in f.blocks:
            lst = b.instructions
            i = 0
            while i < len(lst):
                inst = lst[i]
                si = getattr(inst, "sync_info", None)
                if si is None:
                    i += 1
                    continue
                w = list(si.on_wait or [])
                u = list(si.on_update or [])
                assert len(u) <= 1, (inst.name, u)
                uids = {x.id for x in u}
                conflict = any(x.id in uids for x in w) or (
                    w and any(x.update_mode == "sem-add-imm" for x in u))
                if len(w) <= 1 and not conflict:
                    i += 1
                    continue
                if (w and w[-1].id not in uids
                        and not any(x.update_mode == "sem-add-imm" for x in u)):
                    move, keep = w[:-1], w[-1:]
                else:
                    move, keep = w, []
                for wt in move:
                    nop = mybir.InstNoOp(
                        name=f"{inst.name}-ss{i}", text_hint="syncsplit")
                    nop.engine = inst.engine
                    nop.sync_info = mybir.SyncInfo(on_wait=[wt], on_update=[])
                    nc.register_instruction(nop, overwrite=True)
                    lst.insert(i, nop)
                    i += 1
                inst.sync_info = mybir.SyncInfo(on_wait=keep, on_update=u)
                i += 1


def host_gather(mask_shard: np.ndarray):
    """maskD[j', i, yp, di, dj, px] = mask[di*5+dj, 2i+yp, 2j'-2dj+4+px] (0 OOB)."""
    kk, ny, sw = mask_shard.shape
    ni = ny // 2
    m = mask_shard.reshape(K, K, ni, 2, sw)  # [di, dj, i, yp, x]
    d = np.zeros((128, ni, 2, K, K, 2), dtype=np.float32)
    for dj in range(K):
        for px in range(2):
            x = 2 * np.arange(128) - 2 * dj + 4 + px  # [128]
            valid = (x >= 0) & (x < sw)
            xc = np.clip(x, 0, sw - 1)
            sel = m[:, dj][:, :, :, xc]               # [di, i, yp, 128]
            sel = sel * valid[None, None, None, :]
            d[:, :, :, :, dj, px] = sel.transpose(3, 1, 2, 0)
    return np.ascontiguousarray(d)


def host_bidx():
    """Static scatter index tables (di 0..2 triple | di 3..4 pair) into a
    [K*SW fp32] dst row viewed as uint16."""
    def table(dis):
        idx = np.full((128, len(dis), K, 2, 2), -1, dtype=np.int16)
        for j in range(128):
            for dr, _ in enumerate(dis):
                for dj in range(K):
                    for px in range(2):
                        x = 2 * j - 2 * dj + 4 + px
                        if 0 <= x < SW:
                            idx[j, dr, dj, px, 0] = dr * 2 * SW + 2 * x
                            idx[j, dr, dj, px, 1] = dr * 2 * SW + 2 * x + 1
        return idx.reshape(128, -1)

    return np.ascontiguousarray(
        np.concatenate([table([0, 1, 2]), table([3, 4])], axis=1))


def build_program(n_i: int = N_I, r_in: int = R_IN, relocate: bool = True,
                  detect_races: bool = False):
    nc = bass.Bass(detect_race_conditions=detect_races)

    featt = nc.dram_tensor("featt", [128, r_in, C], F32, kind="ExternalInput")
    maskd = nc.dram_tensor(
        "maskd", [128, n_i, 2 * K * K * 2], F32, kind="ExternalInput"
    )
    bidx = nc.dram_tensor("bidx", [128, NSL], I16, kind="ExternalInput")
    out = nc.dram_tensor("out", [C, 2 * n_i, SW], F32, kind="ExternalOutput")

    assert r_in % RCH == 0
    groups = []

    with tile.TileContext(nc) as tc:
        with (
            tc.tile_pool(name="const", bufs=1) as constp,
            tc.tile_pool(name="ft", bufs=1) as ftp,
            tc.tile_pool(name="maskd", bufs=1) as mdp,
            tc.tile_pool(name="btile", bufs=4) as bp,
            tc.tile_pool(name="orow", bufs=3) as orowp,
            tc.tile_pool(name="mm", bufs=6, space="PSUM") as mmp,
        ):
            nc.gpsimd.load_library(library_config.local_scatter)
            bix = constp.tile([128, NSL], I16, tag="bix")
            nc.sync.dma_start(out=bix[:], in_=bidx[:])

            # maskD resident: [j', i, (yp, di, dj, px)] fp32
            md = mdp.tile([128, n_i, 2 * K * K * 2], F32)
            nc.sync.dma_start(out=md[:], in_=maskd[:])

            # ---- transposed feature rows ft[j', r, c] (host-pretransposed) ----
            ft = ftp.tile([128, r_in, C], F32)
            nc.sync.dma_start(out=ft[:], in_=featt[:])

            # ---- main loop over output row pairs ----
            IB = YB // 2
            for ib0 in range(0, n_i, IB):
                orow = orowp.tile([128, YB, 2, SW], F32, tag="orow")
                for ii in range(IB):
                    i = ib0 + ii
                    # B tiles for this row pair: [j', yp, (di, x) + pad]
                    bt = bp.tile([128, 2, BTP], F32, tag="bt")
                    pre = nc.gpsimd.memset(bt[:, 0, K * SW:BTP], 0.0)
                    if groups:
                        groups[-1][2] = pre  # pre also carries prev group's updates
                        _add_dep(_mi(pre), _mi(groups[-1][1][-1]), sync=False,
                                 reason="chain")
                    prev = pre
                    scats = []
                    for yp in range(2):
                        for lo, hi, i0, i1 in ((0, 3, 0, NSL3), (3, K, NSL3, NSL)):
                            sc = nc.gpsimd.local_scatter(
                                out_ap=bt[:, yp, lo * SW:hi * SW].bitcast(U16),
                                data_ap=md[
                                    :, i,
                                    (yp * K + lo) * K * 2:(yp * K + hi) * K * 2
                                ].bitcast(U16),
                                idxs_ap=bix[:, i0:i1],
                                channels=128,
                                num_elems=(hi - lo) * 2 * SW,
                                num_idxs=i1 - i0,
                            )
                            _add_dep(_mi(sc), _mi(prev), sync=False,
                                     reason="chain")
                            scats.append(sc)
                            prev = sc
                    groups.append([pre, scats, None])

                    for ch in range(2):
                        pm = mmp.tile([128, 2, SW], F32, tag="mm")
                        for di in range(K):
                            nc.tensor.matmul(
                                pm[:],
                                ft[:, i + di, ch * 128:(ch + 1) * 128],
                                bt[:, :, di * SW:(di + 1) * SW],
                                start=(di == 0),
                                stop=(di == K - 1),
                            )
                        if ch == 0:
                            nc.scalar.copy(
                                out=orow[:, 2 * ii:2 * ii + 2, ch, :],
                                in_=pm[:],
                            )
                        else:
                            nc.vector.tensor_copy(
                                orow[:, 2 * ii:2 * ii + 2, ch, :], pm[:],
                            )
                for ch in range(2):
                    dma_eng = nc.sync if ch == 0 else nc.scalar
                    dma_eng.dma_start(
                        out=out[ch * 128:(ch + 1) * 128,
                                2 * ib0:2 * ib0 + YB, :],
                        in_=orow[:, :, ch, :],
                    )
            term = nc.gpsimd.memset(bt[:, 1, K * SW:BTP], 0.0)
            _add_dep(_mi(term), _mi(groups[-1][1][-1]), sync=False,
                     reason="chain")
            groups[-1][2] = term

    if relocate:
        for pre, scats, post in groups:
            relocate_sync([pre], scats, [post])
        split_sync(nc)
    return nc


def finalize_for_hw(nc):
    assert mybir.codegen_inst_isa_subclasses(nc)
    return nc


_PROGRAM = None


def _get_program():
    global _PROGRAM
    if _PROGRAM is None:
        _PROGRAM = finalize_for_hw(build_program())
    return _PROGRAM


def kernel(features: np.ndarray, masks: np.ndarray) -> np.ndarray:
    from concourse.bass_utils import run_bass_kernel_spmd

    features = np.ascontiguousarray(features, dtype=np.float32)
    masks = np.ascontiguousarray(masks, dtype=np.float32)
    fpad = np.pad(features, ((0, 0), (0, 0), (PAD, PAD), (0, 0)))
    bix = host_bidx()

    in_maps = []
    for core in range(N_CORES):
        n, q = divmod(core, 4)
        ftt = fpad[n, :, QH * q:QH * q + R_IN, :].transpose(2, 1, 0)
        in_maps.append({
            "featt": np.ascontiguousarray(ftt),
            "maskd": host_gather(
                masks[n, :, 2 * N_I * q:2 * N_I * (q + 1), :]
            ).reshape(128, N_I, 2 * K * K * 2),
            "bidx": bix,
        })

    nc = _get_program()
    trace = os.environ.get("CARAFE_TRACE") == "1"
    res = run_bass_kernel_spmd(nc, in_maps, list(range(N_CORES)), trace=trace)
    kernel.last_results = res

    out = np.empty((N, C, SH, SW), dtype=np.float32)
    for core in range(N_CORES):
        n, q = divmod(core, 4)
        out[n, :, 2 * N_I * q:2 * N_I * (q + 1), :] = res.results[core]["out"]
    return out



# revision 5
# speedup vs baseline: 3.1291x; 3.1291x over previous
"""CARAFE v2: fp16 banded-matmul on the tensor engine with a split
banded-mask build.

out[c, y, x] = sum_di sum_dj fpad[c, y//2+di, x//2+dj] * m[di*5+dj, y, x]

For a fixed input row index i (covering output rows y=2i and 2i+1) and tap
row di, the contribution over all (yp, x) is a matmul contracting over the
padded input column j' (128 lanes):

    out_i[c, (yp, x)] += sum_{j'} ftT[j', r=i+di, c] * B_di[j', (yp, x)]

where B_di[j', yp, x] = m[(di, dj), 2i+yp, x] at dj = j' - x//2 + 2 (banded,
5 diagonals, zeros elsewhere).  All 5 di-taps accumulate in PSUM.

Everything is fp16 (tolerance 2e-2; fp16 lands ~1e-3): fp16 matmuls stream
at 1 col/cycle vs fp32's 1/4 rate, and halve the banded-build cost.  The
banded tiles are built two ways to balance engines:
  - di in {0,1}: dense banded slabs prebuilt on the host, DMA'd per 4-i
    chunk (trades GPSIMD time for DMA bandwidth),
  - di in {2,3,4}: one GPSIMD local_scatter per i (both yp rows merged into
    a single [128, 1536] u16 region; static index table since the banded
    slot positions are y-independent).

The local_scatter extended-ISA instruction cannot carry semaphore ops through
this walrus build, so its sync is relocated onto adjacent Pool-engine memsets
(sound: Q7 execution is strict FIFO per engine), and a final pass splits any
instruction with more than one wait into standalone sequencer NOPs.
"""

import os

import numpy as np

import concourse.bass as bass
import concourse.mybir as mybir
import concourse.tile as tile
from concourse import library_config

F32 = mybir.dt.float32
F16 = mybir.dt.float16
U16 = mybir.dt.uint16
I16 = mybir.dt.int16
_add_dep = bass._add_dep_helper

N, C, H, W = 2, 256, 128, 128
K = 5
S = 2
PAD = K // 2
SH, SW = H * S, W * S

N_CORES = 8
QH = H // 4          # 32 input rows per core
R_IN = QH + 2 * PAD  # 36 padded feature rows per core
N_I = QH             # 32 output row-pairs per core
YB = 8               # y rows per output DMA batch (4 i's)
IB = YB // 2         # i's per batch

DMA_DI = (0, 1)      # di slabs DMA'd from host-built dense banded tensor
SC_DI = (2, 3, 4)    # di slabs built by GPSIMD local_scatter
ND = len(DMA_DI)
NS = len(SC_DI)
NSL = 2 * NS * K * 2          # scatter slots per partition: yp*di*dj*px = 60
BT_W = 2 * NS * SW            # scatter dst region in u16 = 1536
BT_PAD = 4                    # sync-carrier pad columns


def _mi(x):
    return getattr(x, "ins", x)


def relocate_sync(pres, scats, posts):
    """Move the scatters' semaphore waits onto `pres` and updates onto
    `posts` (all chained in Pool-engine program order via nosync deps; Q7
    execution is strict FIFO per engine, so advancing waits and delaying
    updates across the group is sync-preserving).  Waits merge by max per
    semaphore, updates merge by sum."""
    def si_of(inst):
        si = inst.sync_info
        if si is None:
            return [], []
        return list(si.on_wait or []), list(si.on_update or [])

    wmax, uacc = {}, {}
    for s in scats:
        w, u = si_of(_mi(s))
        for x in w:
            assert x.sync_type == "semaphore" and x.wait_mode == "sem-ge-imm", x
            prev = wmax.get(x.id)
            if prev is None or x.wait_value > prev.wait_value:
                wmax[x.id] = x
        for x in u:
            assert x.sync_type == "semaphore" and x.update_mode in (
                "sem-inc", "sem-add-imm"), x
            prev = uacc.get(x.id)
            if prev is None:
                uacc[x.id] = mybir.SyncUpdate(
                    sync_type="semaphore", id=x.id, ant_name=x.ant_name,
                    update_mode="sem-add-imm", update_value=x.update_value)
            else:
                prev.update_value = prev.update_value + x.update_value
        _mi(s).sync_info = mybir.SyncInfo(on_wait=[], on_update=[])

    for carrier in pres:
        ci = _mi(carrier)
        cw, cu = si_of(ci)
        for w in cw:
            inc = wmax.pop(w.id, None)
            if inc is not None and inc.wait_value > w.wait_value:
                w.wait_value = inc.wait_value
        take = list(wmax.values())
        wmax.clear()
        ci.sync_info = mybir.SyncInfo(on_wait=cw + take, on_update=cu)
        break
    assert not wmax

    for carrier in posts:
        ci = _mi(carrier)
        cw, cu = si_of(ci)
        for u in cu:
            inc = uacc.pop(u.id, None)
            if inc is not None:
                u.update_value = u.update_value + inc.update_value
                u.update_mode = "sem-add-imm"
        take = list(uacc.values())
        uacc.clear()
        ci.sync_info = mybir.SyncInfo(on_wait=cw, on_update=cu + take)
        break
    assert not uacc


def split_sync(nc):
    """Enforce <=1 wait and <=1 update per instruction (this walrus build's
    events capacity), hoisting excess waits onto standalone same-engine
    sequencer NOPs placed immediately before (sync-equivalent).  Also hoists
    a wait that shares its semaphore with the instruction's own update."""
    for f in nc.m.functions:
        for b in f.blocks:
            lst = b.instructions
            i = 0
            while i < len(lst):
                inst = lst[i]
                si = getattr(inst, "sync_info", None)
                if si is None:
                    i += 1
                    continue
                w = list(si.on_wait or [])
                u = list(si.on_update or [])
                assert len(u) <= 1, (inst.name, u)
                uids = {x.id for x in u}
                conflict = any(x.id in uids for x in w) or (
                    w and any(x.update_mode == "sem-add-imm" for x in u))
                if len(w) <= 1 and not conflict:
                    i += 1
                    continue
                if (w and w[-1].id not in uids
                        and not any(x.update_mode == "sem-add-imm" for x in u)):
                    move, keep = w[:-1], w[-1:]
                else:
                    move, keep = w, []
                for wt in move:
                    nop = mybir.InstNoOp(
                        name=f"{inst.name}-ss{i}", text_hint="syncsplit")
                    nop.engine = inst.engine
                    nop.sync_info = mybir.SyncInfo(on_wait=[wt], on_update=[])
                    nc.register_instruction(nop, overwrite=True)
                    lst.insert(i, nop)
                    i += 1
                inst.sync_info = mybir.SyncInfo(on_wait=keep, on_update=u)
                i += 1


def host_maskd(mask_shard: np.ndarray):
    """Scatter payload for di in SC_DI:
    md[j', i, (yp, dr, dj, px)] = m[(SC_DI[dr], dj), 2i+yp, 2j'-2dj+4+px]
    (0 when the target column is OOB; those slots have idx -1)."""
    kk, ny, sw = mask_shard.shape
    ni = ny // 2
    mv = mask_shard.reshape(K, K, ni, 2, sw)  # [di, dj, i, yp, x]
    d = np.zeros((128, ni, 2, NS, K, 2), dtype=np.float16)
    for dj in range(K):
        for px in range(2):
            x = 2 * np.arange(128) - 2 * dj + 4 + px  # [128]
            valid = (x >= 0) & (x < sw)
            xc = np.clip(x, 0, sw - 1)
            for dr, di in enumerate(SC_DI):
                sel = mv[di, dj][:, :, xc]            # [i, yp, 128]
                sel = sel * valid[None, None, :]
                d[:, :, :, dr, dj, px] = sel.transpose(2, 0, 1)
    return np.ascontiguousarray(d.reshape(128, ni, NSL))


def host_bidx():
    """Static scatter index table into the [2*NS*SW u16] dst region
    (layout [yp, dr, x]); -1 marks OOB slots (ignored by the scatter)."""
    idx = np.full((128, 2, NS, K, 2), -1, dtype=np.int16)
    for j in range(128):
        for yp in range(2):
            for dr in range(NS):
                for dj in range(K):
                    for px in range(2):
                        x = 2 * j - 2 * dj + 4 + px
                        if 0 <= x < SW:
                            idx[j, yp, dr, dj, px] = (yp * NS + dr) * SW + x
    return np.ascontiguousarray(idx.reshape(128, NSL))


def host_bdma(mask_shard: np.ndarray):
    """Dense banded slabs for di in DMA_DI:
    bd[j', i, yp, dd, x] = m[(DMA_DI[dd], dj), 2i+yp, x] at dj = j'-x//2+2,
    zero outside the band."""
    kk, ny, sw = mask_shard.shape
    ni = ny // 2
    mv = mask_shard.reshape(K, K, ni, 2, sw)  # [di, dj, i, yp, x]
    bd = np.zeros((128, ni, 2, ND, sw), dtype=np.float16)
    for dj in range(K):
        for px in range(2):
            x = 2 * np.arange(128) - 2 * dj + 4 + px  # [128] per j'
            valid = (x >= 0) & (x < sw)
            jv = np.arange(128)[valid]
            xv = x[valid]
            for dd, di in enumerate(DMA_DI):
                # [i, yp, nv] -> [nv, i, yp]
                bd[jv, :, :, dd, xv] = mv[di, dj][:, :, xv].transpose(2, 0, 1)
    return np.ascontiguousarray(bd)


def build_program(n_i: int = N_I, r_in: int = R_IN, relocate: bool = True,
                  detect_races: bool = False):
    nc = bass.Bass(detect_race_conditions=detect_races)

    featt = nc.dram_tensor("featt", [128, r_in, C], F16, kind="ExternalInput")
    maskd = nc.dram_tensor("maskd", [128, n_i, NSL], F16, kind="ExternalInput")
    bidx = nc.dram_tensor("bidx", [128, NSL], I16, kind="ExternalInput")
    bdma = nc.dram_tensor(
        "bdma", [128, n_i, 2, ND, SW], F16, kind="ExternalInput")
    out = nc.dram_tensor("out", [C, 2 * n_i, SW], F32, kind="ExternalOutput")

    groups = []

    with tile.TileContext(nc) as tc:
        with (
            tc.tile_pool(name="const", bufs=1) as constp,
            tc.tile_pool(name="ft", bufs=1) as ftp,
            tc.tile_pool(name="maskd", bufs=1) as mdp,
            tc.tile_pool(name="bc", bufs=3) as bcp,
            tc.tile_pool(name="btile", bufs=4) as bp,
            tc.tile_pool(name="orow", bufs=3) as orowp,
            tc.tile_pool(name="mm", bufs=6, space="PSUM") as mmp,
        ):
            nc.gpsimd.load_library(library_config.local_scatter)
            bix = constp.tile([128, NSL], I16, tag="bix")
            nc.sync.dma_start(out=bix[:], in_=bidx[:])
            # sync-carrier anchor for the relocated scatter semaphores
            panchor = constp.tile([128, BT_PAD], F16, tag="panchor")

            # scatter payload resident: [j', i, (yp, dr, dj, px)] fp16
            md = mdp.tile([128, n_i, NSL], F16)
            nc.sync.dma_start(out=md[:], in_=maskd[:])

            # transposed feature rows ft[j', r, c] (host-pretransposed) fp16
            ft = ftp.tile([128, r_in, C], F16)
            nc.sync.dma_start(out=ft[:], in_=featt[:])

            # ---- main loop over output row pairs ----
            for ib0 in range(0, n_i, IB):
                # host-built dense banded slabs for di in DMA_DI, 4 i's/chunk
                bc = bcp.tile([128, IB, 2, ND, SW], F16, tag="bc")
                nc.scalar.dma_start(
                    out=bc[:], in_=bdma[:, ib0:ib0 + IB])
                orow = orowp.tile([128, YB, 2, SW], F32, tag="orow")
                for ii in range(IB):
                    i = ib0 + ii
                    # scatter region [yp, dr, x]
                    bt = bp.tile([128, 2, NS, SW], F16, tag="bt")
                    pre = nc.gpsimd.memset(panchor[:], 0.0)
                    if groups:
                        groups[-1][2] = pre  # pre also carries prev updates
                        _add_dep(_mi(pre), _mi(groups[-1][1][-1]), sync=False,
                                 reason="chain")
                    sc = nc.gpsimd.local_scatter(
                        out_ap=bt[:].bitcast(U16),
                        data_ap=md[:, i, :].bitcast(U16),
                        idxs_ap=bix[:],
                        channels=128,
                        num_elems=BT_W,
                        num_idxs=NSL,
                    )
                    _add_dep(_mi(sc), _mi(pre), sync=False, reason="chain")
                    groups.append([pre, [sc], None])

                    btv = bt  # [128, 2, NS, SW]
                    bcv = bc[:, ii]  # [128, 2, ND, SW]
                    for ch in range(2):
                        pm = mmp.tile([128, 2, SW], F32, tag="mm")
                        for dd, di in enumerate(DMA_DI):
                            nc.tensor.matmul(
                                pm[:],
                                ft[:, i + di, ch * 128:(ch + 1) * 128],
                                bcv[:, :, dd, :],
                                start=(di == 0),
                                stop=False,
                            )
                        for dr, di in enumerate(SC_DI):
                            nc.tensor.matmul(
                                pm[:],
                                ft[:, i + di, ch * 128:(ch + 1) * 128],
                                btv[:, :, dr, :],
                                start=False,
                                stop=(di == K - 1),
                            )
                        if ch == 0:
                            nc.scalar.copy(
                                out=orow[:, 2 * ii:2 * ii + 2, ch, :],
                                in_=pm[:],
                            )
                        else:
                            nc.vector.tensor_copy(
                                orow[:, 2 * ii:2 * ii + 2, ch, :], pm[:],
                            )
                for ch in range(2):
                    dma_eng = nc.sync if ch == 0 else nc.scalar
                    dma_eng.dma_start(
                        out=out[ch * 128:(ch + 1) * 128,
                                2 * ib0:2 * ib0 + YB, :],
                        in_=orow[:, :, ch, :],
                    )
            term = nc.gpsimd.memset(panchor[:], 0.0)
            _add_dep(_mi(term), _mi(groups[-1][1][-1]), sync=False,
                     reason="chain")
            groups[-1][2] = term

    if relocate:
        for pre, scats, post in groups:
            relocate_sync([pre], scats, [post])
        split_sync(nc)
    return nc


def finalize_for_hw(nc):
    assert mybir.codegen_inst_isa_subclasses(nc)
    return nc


_PROGRAM = None


def _get_program():
    global _PROGRAM
    if _PROGRAM is None:
        _PROGRAM = finalize_for_hw(build_program())
    return _PROGRAM


def kernel(features: np.ndarray, masks: np.ndarray) -> np.ndarray:
    from concourse.bass_utils import run_bass_kernel_spmd

    features = np.ascontiguousarray(features, dtype=np.float32)
    masks = np.ascontiguousarray(masks, dtype=np.float32)
    fpad = np.pad(features, ((0, 0), (0, 0), (PAD, PAD), (0, 0)))
    bix = host_bidx()

    in_maps = []
    for core in range(N_CORES):
        n, q = divmod(core, 4)
        ftt = fpad[n, :, QH * q:QH * q + R_IN, :].transpose(2, 1, 0)
        mshard = masks[n, :, 2 * N_I * q:2 * N_I * (q + 1), :]
        in_maps.append({
            "featt": np.ascontiguousarray(ftt, dtype=np.float16),
            "maskd": host_maskd(mshard),
            "bidx": bix,
            "bdma": host_bdma(mshard),
        })

    nc = _get_program()
    trace = os.environ.get("CARAFE_TRACE") == "1"
    res = run_bass_kernel_spmd(nc, in_maps, list(range(N_CORES)), trace=trace)
    kernel.last_results = res

    out = np.empty((N, C, SH, SW), dtype=np.float32)
    for core in range(N_CORES):
        n, q = divmod(core, 4)
        out[n, :, 2 * N_I * q:2 * N_I * (q + 1), :] = res.results[core]["out"]
    return out


# revision 43
# speedup vs baseline: 4.1681x; 1.3321x over previous
"""CARAFE v3: fp16 banded-matmul on the tensor engine, PE-bound.

out[c, y, x] = sum_di sum_dj fpad[c, y//2+di, x//2+dj] * m[di*5+dj, y, x]

For a fixed input row index i (covering output rows y=2i and 2i+1) and tap
row di, the contribution over all (yp, x) is a matmul contracting over the
padded input column j' (128 lanes):

    out_i[c, (yp, x)] += sum_{j'} ftT[j', r=i+di, c] * B_di[j', (yp, x)]

where B_di[j', yp, x] = m[(di, dj), 2i+yp, x] at dj = j' - x//2 + 2 (banded,
5 diagonals, zeros elsewhere).  All 5 di-taps accumulate in PSUM.

Everything on-chip is fp16 (harness tolerance 2e-2; fp16 lands ~6e-4):
fp16 matmuls stream at 1 col/cycle vs fp32's 1/4 rate, the banded-build
halves, and the output DMA halves (fp16 out, host casts back to fp32).
The banded tiles are built two ways to balance engines:
  - di in {0,1,2}: dense banded slabs prebuilt on the host, DMA'd per 4-i
    chunk (trades GPSIMD time for DMA bandwidth),
  - di in {3,4}: one GPSIMD local_scatter per i (both yp rows merged into
    a single [128, 1024] u16 region; static index table since the banded
    slot positions are y-independent).
Steady state is tensor-engine-bound (~2.13us per row pair); Pool (~1.5us)
and DMA (~2.08us) stay ahead so the PE never idles (idle gaps reset the
cost model's p-state ramp).

The local_scatter extended-ISA instruction cannot carry semaphore ops through
this walrus build, so its sync is relocated onto adjacent Pool-engine memsets
(sound: Q7 execution is strict FIFO per engine), and a final pass splits any
instruction with more than one wait into standalone sequencer NOPs.
"""

import os

import numpy as np

import concourse.bass as bass
import concourse.mybir as mybir
import concourse.tile as tile
from concourse import library_config

F32 = mybir.dt.float32
F16 = mybir.dt.float16
U16 = mybir.dt.uint16
I16 = mybir.dt.int16
_add_dep = bass._add_dep_helper

N, C, H, W = 2, 256, 128, 128
K = 5
S = 2
PAD = K // 2
SH, SW = H * S, W * S

N_CORES = 8
QH = H // 4          # 32 input rows per core
R_IN = QH + 2 * PAD  # 36 padded feature rows per core
N_I = QH             # 32 output row-pairs per core
YB = 8               # y rows per output DMA batch (4 i's)
IB = YB // 2         # i's per batch

# The banded tiles are split between a host-built DMA'd tensor and the
# on-chip GPSIMD scatter, alternating by row-pair parity so that both the
# DMA engines and the Pool engine stay just under the tensor engine's
# steady-state rate:  even i: DMA {0,1,2} + scatter {3,4};
#                     odd  i: DMA {0,1}   + scatter {2,3,4}.
# The first N_LP pairs use the light scatter for BOTH rows (their odd-di2
# slab is DMA'd too, bdma slot 5) so the Pool engine builds phase margin
# before the heavy scatters start.
SC_E = (3, 4)        # scattered di, even i (and odd i of light pairs)
SC_O = (2, 3, 4)     # scattered di, odd i of heavy pairs
NSL_E = 2 * len(SC_E) * K * 2   # 40 scatter slots per partition, light
NSL_O = 2 * len(SC_O) * K * 2   # 60, heavy
N_LP = 0             # light pairs
N_WARM = 9           # PE warmup matmuls (cover startup DMA latency)
N_WARM_SM = 5        # small trailing warmups (fine-grained alignment)


def md_off(p):
    """maskd column offset of pair p (light pairs hold 80 slots, heavy 100)."""
    return 2 * NSL_E * p if p < N_LP else (
        2 * NSL_E * N_LP + (NSL_E + NSL_O) * (p - N_LP))


def _mi(x):
    return getattr(x, "ins", x)


def relocate_sync(pres, scats, posts):
    """Move the scatters' semaphore waits onto `pres` and updates onto
    `posts` (all chained in Pool-engine program order via nosync deps; Q7
    execution is strict FIFO per engine, so advancing waits and delaying
    updates across the group is sync-preserving).  Waits merge by max per
    semaphore, updates merge by sum."""
    def si_of(inst):
        si = inst.sync_info
        if si is None:
            return [], []
        return list(si.on_wait or []), list(si.on_update or [])

    wmax, uacc = {}, {}
    for s in scats:
        w, u = si_of(_mi(s))
        for x in w:
            assert x.sync_type == "semaphore" and x.wait_mode == "sem-ge-imm", x
            prev = wmax.get(x.id)
            if prev is None or x.wait_value > prev.wait_value:
                wmax[x.id] = x
        for x in u:
            assert x.sync_type == "semaphore" and x.update_mode in (
                "sem-inc", "sem-add-imm"), x
            prev = uacc.get(x.id)
            if prev is None:
                uacc[x.id] = mybir.SyncUpdate(
                    sync_type="semaphore", id=x.id, ant_name=x.ant_name,
                    update_mode="sem-add-imm", update_value=x.update_value)
            else:
                prev.update_value = prev.update_value + x.update_value
        _mi(s).sync_info = mybir.SyncInfo(on_wait=[], on_update=[])

    for carrier in pres:
        ci = _mi(carrier)
        cw, cu = si_of(ci)
        for w in cw:
            inc = wmax.pop(w.id, None)
            if inc is not None and inc.wait_value > w.wait_value:
                w.wait_value = inc.wait_value
        take = list(wmax.values())
        wmax.clear()
        ci.sync_info = mybir.SyncInfo(on_wait=cw + take, on_update=cu)
        break
    assert not wmax

    for carrier in posts:
        ci = _mi(carrier)
        cw, cu = si_of(ci)
        for u in cu:
            inc = uacc.pop(u.id, None)
            if inc is not None:
                u.update_value = u.update_value + inc.update_value
                u.update_mode = "sem-add-imm"
        take = list(uacc.values())
        uacc.clear()
        ci.sync_info = mybir.SyncInfo(on_wait=cw, on_update=cu + take)
        break
    assert not uacc


def split_sync(nc):
    """Enforce <=1 wait and <=1 update per instruction (this walrus build's
    events capacity), hoisting excess waits onto standalone same-engine
    sequencer NOPs placed immediately before (sync-equivalent).  Also hoists
    a wait that shares its semaphore with the instruction's own update."""
    for f in nc.m.functions:
        for b in f.blocks:
            lst = b.instructions
            i = 0
            while i < len(lst):
                inst = lst[i]
                si = getattr(inst, "sync_info", None)
                if si is None:
                    i += 1
                    continue
                w = list(si.on_wait or [])
                u = list(si.on_update or [])
                assert len(u) <= 1, (inst.name, u)
                uids = {x.id for x in u}
                conflict = any(x.id in uids for x in w) or (
                    w and any(x.update_mode == "sem-add-imm" for x in u))
                if len(w) <= 1 and not conflict:
                    i += 1
                    continue
                if (w and w[-1].id not in uids
                        and not any(x.update_mode == "sem-add-imm" for x in u)):
                    move, keep = w[:-1], w[-1:]
                else:
                    move, keep = w, []
                for wt in move:
                    nop = mybir.InstNoOp(
                        name=f"{inst.name}-ss{i}", text_hint="syncsplit")
                    nop.engine = inst.engine
                    nop.sync_info = mybir.SyncInfo(on_wait=[wt], on_update=[])
                    nc.register_instruction(nop, overwrite=True)
                    lst.insert(i, nop)
                    i += 1
                inst.sync_info = mybir.SyncInfo(on_wait=keep, on_update=u)
                i += 1


def _slots(mask_shard, i, scs):
    """Scatter payload for row-pair i over the di set `scs`:
    [j', (yp, dr, dj, px)] = m[(scs[dr], dj), 2i+yp, 2j'-2dj+4+px]
    (0 when the target column is OOB; those slots have idx -1)."""
    kk, ny, sw = mask_shard.shape
    mv = mask_shard.reshape(K, K, ny // 2, 2, sw)  # [di, dj, i, yp, x]
    d = np.zeros((128, 2, len(scs), K, 2), dtype=np.float16)
    for dj in range(K):
        for px in range(2):
            x = 2 * np.arange(128) - 2 * dj + 4 + px  # [128]
            valid = (x >= 0) & (x < sw)
            xc = np.clip(x, 0, sw - 1)
            for dr, di in enumerate(scs):
                sel = mv[di, dj, i][:, xc] * valid[None, :]  # [yp, 128]
                d[:, :, dr, dj, px] = sel.T
    return d.reshape(128, -1)


def host_maskd(mask_shard: np.ndarray):
    """Concatenated scatter payloads: per pair, even-i slots then odd-i
    slots ({3,4} for light pairs, {2,3,4} for heavy)."""
    ni = mask_shard.shape[1] // 2
    parts = []
    for p in range(ni // 2):
        parts.append(_slots(mask_shard, 2 * p, SC_E))
        parts.append(_slots(mask_shard, 2 * p + 1,
                            SC_E if p < N_LP else SC_O))
    return np.ascontiguousarray(np.concatenate(parts, axis=1))


def host_bidx():
    """Static scatter index tables into the [2*ns*SW u16] dst regions
    (layout [yp, dr, x]); -1 marks OOB slots (ignored by the scatter).
    Concatenated [even (40) | odd (60)] -> [128, 100]."""
    def table(scs):
        ns = len(scs)
        idx = np.full((128, 2, ns, K, 2), -1, dtype=np.int16)
        for j in range(128):
            for yp in range(2):
                for dr in range(ns):
                    for dj in range(K):
                        for px in range(2):
                            x = 2 * j - 2 * dj + 4 + px
                            if 0 <= x < SW:
                                idx[j, yp, dr, dj, px] = (yp * ns + dr) * SW + x
        return idx.reshape(128, -1)

    return np.ascontiguousarray(
        np.concatenate([table(SC_E), table(SC_O)], axis=1))


def host_bdma(mask_shard: np.ndarray):
    """Dense banded slabs per i-pair: bd[j', p, yp, slot, x] with
    slots 0..2 = even-i di {0,1,2}, slots 3..4 = odd-i di {0,1},
    slot 5 = odd-i di 2 (filled for all pairs, DMA'd only for light ones):
    value = m[(di, dj), y, x] at dj = j'-x//2+2, zero outside the band."""
    kk, ny, sw = mask_shard.shape
    ni = ny // 2
    mv = mask_shard.reshape(K, K, ni, 2, sw)  # [di, dj, i, yp, x]
    bd = np.zeros((128, ni // 2, 2, 6, sw), dtype=np.float16)
    for dj in range(K):
        for px in range(2):
            x = 2 * np.arange(128) - 2 * dj + 4 + px  # [128] per j'
            valid = (x >= 0) & (x < sw)
            jv = np.arange(128)[valid]
            xv = x[valid]
            for slot, (par, di) in enumerate(
                    [(0, 0), (0, 1), (0, 2), (1, 0), (1, 1), (1, 2)]):
                # [ni/2, yp, nv] -> [nv, ni/2, yp]
                bd[jv, :, :, slot, xv] = (
                    mv[di, dj][par::2][:, :, xv].transpose(2, 0, 1))
    return np.ascontiguousarray(bd)


def build_program(n_i: int = N_I, r_in: int = R_IN, relocate: bool = True,
                  detect_races: bool = False):
    nc = bass.Bass(detect_race_conditions=detect_races)

    md_total = md_off(n_i // 2)
    featt = nc.dram_tensor("featt", [128, r_in, C], F16, kind="ExternalInput")
    # flat [j', pair-slots] so DMA descriptors stay >=512B contiguous
    maskd = nc.dram_tensor("maskd", [128, md_total], F16,
                           kind="ExternalInput")
    bidx = nc.dram_tensor(
        "bidx", [128, NSL_E + NSL_O], I16, kind="ExternalInput")
    bdma = nc.dram_tensor(
        "bdma", [128, n_i // 2, 2, 6, SW], F16, kind="ExternalInput")
    out = nc.dram_tensor("out", [C, 2 * n_i, SW], F16, kind="ExternalOutput")

    groups = []

    with tile.TileContext(nc) as tc:
        with (
            tc.tile_pool(name="const", bufs=1) as constp,
            tc.tile_pool(name="ft", bufs=1) as ftp,
            tc.tile_pool(name="maskd", bufs=1) as mdp,
            tc.tile_pool(name="bc", bufs=7) as bcp,
            tc.tile_pool(name="btile", bufs=4) as bp,
            tc.tile_pool(name="orow", bufs=4) as orowp,
            tc.tile_pool(name="mm", bufs=6, space="PSUM") as mmp,
            tc.tile_pool(name="mmwarm", bufs=1, space="PSUM") as mmwp,
        ):
            nc.gpsimd.load_library(library_config.local_scatter)
            # PE warmup: keep the tensor engine continuously busy from t~0
            # so the p-state ramp completes while the startup DMAs land and
            # every real matmul runs at full clock.  Reads uninitialized
            # SBUF; the warm PSUM bank is never read back.
            warm = constp.tile([128, 512 + 128], F16, tag="warm")
            warm_pm = mmwp.tile([128, 512], F32, tag="mmwarm")
            nc.gpsimd.memset(warm[:], 0.0)
            for _ in range(N_WARM):
                nc.tensor.matmul(
                    warm_pm[:], warm[:, 512:640], warm[:, 0:512],
                    start=True, stop=True)
            for _ in range(N_WARM_SM):
                nc.tensor.matmul(
                    warm_pm[:, 0:128], warm[:, 512:640], warm[:, 0:128],
                    start=True, stop=True)

            # All DMAs issue on SP so their service order follows program
            # order: startup loads critical-path first (scatter table +
            # payload for early i's, feature rows 0-5, banded slabs for
            # i 0-1), then the rest interleaved.
            bix = constp.tile([128, NSL_E + NSL_O], I16, tag="bix")
            nc.sync.dma_start(out=bix[:], in_=bidx[:])
            # sync-carrier anchor for the relocated scatter semaphores
            panchor = constp.tile([128, 4], F16, tag="panchor")

            md = mdp.tile([128, md_total], F16)
            md3 = md_off(3)
            nc.sync.dma_start(out=md[:, :md3], in_=maskd[:, :md3])

            ft = ftp.tile([128, r_in, C], F16)
            bcs = {}

            def load_bc(p):
                bct = bcp.tile([128, 2, 6, SW], F16, tag="bc", name="bc")
                bcs[p] = bct
                nsl = 6 if p < N_LP else 5
                nc.sync.dma_start(
                    out=bct[:, :, 0:nsl, :], in_=bdma[:, p, :, 0:nsl])

            nc.sync.dma_start(out=ft[:, 0:5, :], in_=featt[:, 0:5, :])
            # first pair's slabs split by parity so i=0 can start sooner
            bc0 = bcp.tile([128, 2, 6, SW], F16, tag="bc", name="bc")
            bcs[0] = bc0
            nc.sync.dma_start(out=bc0[:, :, 0:3, :], in_=bdma[:, 0, :, 0:3])
            nc.sync.dma_start(out=ft[:, 5:12, :], in_=featt[:, 5:12, :])
            nc.sync.dma_start(out=bc0[:, :, 3:5, :], in_=bdma[:, 0, :, 3:5])
            load_bc(1)

            # remaining loads are issued inside the loop, positioned by
            # when their consumer runs, so early DMA serves only what is
            # imminent (SP program order == DMA service order).
            md9 = md_off(10)
            deferred = {
                0: [lambda: load_bc(2)],
                2: [lambda: nc.sync.dma_start(
                    out=ft[:, 12:18, :], in_=featt[:, 12:18, :])],
                4: [lambda: nc.sync.dma_start(
                    out=ft[:, 18:24, :], in_=featt[:, 18:24, :])],
                6: [lambda: nc.sync.dma_start(
                    out=ft[:, 24:30, :], in_=featt[:, 24:30, :])],
                8: [lambda: nc.sync.dma_start(
                    out=ft[:, 30:r_in, :], in_=featt[:, 30:r_in, :])],
                1: [lambda: nc.sync.dma_start(
                    out=md[:, md3:md9], in_=maskd[:, md3:md9])],
                7: [lambda: nc.sync.dma_start(
                    out=md[:, md9:], in_=maskd[:, md9:])],
            }
            for oi in range(1, 26, 2):
                deferred.setdefault(oi, []).append(
                    lambda pp=(oi + 5) // 2: load_bc(pp))

            # ---- main loop over output row pairs ----
            for i in range(n_i):
                par = i % 2
                p = i // 2
                for fn in deferred.get(i, ()):
                    fn()
                # scatter region [yp, dr, x]
                heavy = par == 1 and p >= N_LP
                ns = len(SC_O) if heavy else len(SC_E)
                bt = bp.tile([128, 2, ns, SW], F16,
                             tag=f"bt{int(heavy)}", name="bt")
                pre = nc.gpsimd.memset(panchor[:], 0.0)
                if groups:
                    groups[-1][2] = pre  # pre also carries prev updates
                    _add_dep(_mi(pre), _mi(groups[-1][1][-1]), sync=False,
                             reason="chain")
                moff = md_off(p) + (0 if par == 0 else NSL_E)
                nsl = NSL_O if heavy else NSL_E
                sc = nc.gpsimd.local_scatter(
                    out_ap=bt[:].bitcast(U16),
                    data_ap=md[:, moff:moff + nsl].bitcast(U16),
                    idxs_ap=bix[:, NSL_E:NSL_E + NSL_O] if heavy
                    else bix[:, 0:NSL_E],
                    channels=128,
                    num_elems=2 * ns * SW,
                    num_idxs=nsl,
                )
                _add_dep(_mi(sc), _mi(pre), sync=False, reason="chain")
                groups.append([pre, [sc], None])

                orow = orowp.tile([128, 2, 2, SW], F16, tag="orow")
                bcv = bcs[p]  # [128, 2, 6, SW]: slots e0 e1 e2 o0 o1 o2
                nd = 2 if heavy else 3
                slot0 = 0 if par == 0 else 3
                # last row pair: ch0 last, so the final output goes through
                # the single-engine ACT copy+DMA path (shortest tail)
                for ch in ((1, 0) if i == n_i - 1 else (0, 1)):
                    pm = mmp.tile([128, 2, SW], F32, tag="mm")
                    # operands: DMA'd slabs (di < nd) and scattered slabs;
                    # for i=0 the scatter lands first, so emit its matmuls
                    # first (start/stop flags follow emission order)
                    ops = [(di, bcv[:, :, slot0 + di, :])
                           for di in range(nd)]
                    ops += [(nd + dr, bt[:, :, dr, :]) for dr in range(ns)]
                    if i == 0:
                        ops = ops[nd:] + ops[:nd]
                    for k, (di, rhs) in enumerate(ops):
                        nc.tensor.matmul(
                            pm[:],
                            ft[:, i + di, ch * 128:(ch + 1) * 128],
                            rhs,
                            start=(k == 0),
                            stop=(k == K - 1),
                        )
                    if ch == 0:
                        nc.scalar.copy(
                            out=orow[:, :, ch, :], in_=pm[:])
                    else:
                        nc.vector.tensor_copy(
                            orow[:, :, ch, :], pm[:])
                # per-i output DMAs, split across the two HWDGE issue
                # engines (ch0 from ACT right after its copy, ch1 from SP)
                # so their sequencer costs parallelize; the last pair issues
                # both from ACT to keep the tail on one fast path
                engs = ((0, nc.scalar), (1, nc.sync))
                for ch, eng in engs:
                    eng.dma_start(
                        out=out[ch * 128:(ch + 1) * 128, 2 * i:2 * i + 2, :],
                        in_=orow[:, :, ch, :],
                    )
            term = nc.gpsimd.memset(panchor[:], 0.0)
            _add_dep(_mi(term), _mi(groups[-1][1][-1]), sync=False,
                     reason="chain")
            groups[-1][2] = term

    if relocate:
        for pre, scats, post in groups:
            relocate_sync([pre], scats, [post])
        split_sync(nc)
    return nc


def finalize_for_hw(nc):
    assert mybir.codegen_inst_isa_subclasses(nc)
    return nc


_PROGRAM = None


def _get_program():
    global _PROGRAM
    if _PROGRAM is None:
        _PROGRAM = finalize_for_hw(build_program())
    return _PROGRAM


def kernel(features: np.ndarray, masks: np.ndarray) -> np.ndarray:
    from concourse.bass_utils import run_bass_kernel_spmd

    features = np.ascontiguousarray(features, dtype=np.float32)
    masks = np.ascontiguousarray(masks, dtype=np.float32)
    fpad = np.pad(features, ((0, 0), (0, 0), (PAD, PAD), (0, 0)))
    bix = host_bidx()

    in_maps = []
    for core in range(N_CORES):
        n, q = divmod(core, 4)
        ftt = fpad[n, :, QH * q:QH * q + R_IN, :].transpose(2, 1, 0)
        mshard = masks[n, :, 2 * N_I * q:2 * N_I * (q + 1), :]
        in_maps.append({
            "featt": np.ascontiguousarray(ftt, dtype=np.float16),
            "maskd": host_maskd(mshard),
            "bidx": bix,
            "bdma": host_bdma(mshard),
        })

    nc = _get_program()
    trace = os.environ.get("CARAFE_TRACE") == "1"
    res = run_bass_kernel_spmd(nc, in_maps, list(range(N_CORES)), trace=trace)
    kernel.last_results = res

    out = np.empty((N, C, SH, SW), dtype=np.float32)
    for core in range(N_CORES):
        n, q = divmod(core, 4)
        out[n, :, 2 * N_I * q:2 * N_I * (q + 1), :] = (
            res.results[core]["out"].astype(np.float32))
    return out


# revision 48
# speedup vs baseline: 4.1984x; 1.0073x over previous
"""CARAFE v3: fp16 banded-matmul on the tensor engine, PE-bound.

out[c, y, x] = sum_di sum_dj fpad[c, y//2+di, x//2+dj] * m[di*5+dj, y, x]

For a fixed input row index i (covering output rows y=2i and 2i+1) and tap
row di, the contribution over all (yp, x) is a matmul contracting over the
padded input column j' (128 lanes):

    out_i[c, (yp, x)] += sum_{j'} ftT[j', r=i+di, c] * B_di[j', (yp, x)]

where B_di[j', yp, x] = m[(di, dj), 2i+yp, x] at dj = j' - x//2 + 2 (banded,
5 diagonals, zeros elsewhere).  All 5 di-taps accumulate in PSUM.

Everything on-chip is fp16 (harness tolerance 2e-2; fp16 lands ~6e-4):
fp16 matmuls stream at 1 col/cycle vs fp32's 1/4 rate, the banded-build
halves, and the output DMA halves (fp16 out, host casts back to fp32).
The banded tiles are built two ways to balance engines:
  - di in {0,1,2}: dense banded slabs prebuilt on the host, DMA'd per 4-i
    chunk (trades GPSIMD time for DMA bandwidth),
  - di in {3,4}: one GPSIMD local_scatter per i (both yp rows merged into
    a single [128, 1024] u16 region; static index table since the banded
    slot positions are y-independent).
Steady state is tensor-engine-bound (~2.13us per row pair); Pool (~1.5us)
and DMA (~2.08us) stay ahead so the PE never idles (idle gaps reset the
cost model's p-state ramp).

The local_scatter extended-ISA instruction cannot carry semaphore ops through
this walrus build, so its sync is relocated onto adjacent Pool-engine memsets
(sound: Q7 execution is strict FIFO per engine), and a final pass splits any
instruction with more than one wait into standalone sequencer NOPs.
"""

import os

import numpy as np

import concourse.bass as bass
import concourse.mybir as mybir
import concourse.tile as tile
from concourse import library_config

F32 = mybir.dt.float32
F16 = mybir.dt.float16
U16 = mybir.dt.uint16
I16 = mybir.dt.int16
_add_dep = bass._add_dep_helper

N, C, H, W = 2, 256, 128, 128
K = 5
S = 2
PAD = K // 2
SH, SW = H * S, W * S

N_CORES = 8
QH = H // 4          # 32 input rows per core
R_IN = QH + 2 * PAD  # 36 padded feature rows per core
N_I = QH             # 32 output row-pairs per core
YB = 8               # y rows per output DMA batch (4 i's)
IB = YB // 2         # i's per batch

# The banded tiles are split between a host-built DMA'd tensor and the
# on-chip GPSIMD scatter, alternating by row-pair parity so that both the
# DMA engines and the Pool engine stay just under the tensor engine's
# steady-state rate:  even i: DMA {0,1,2} + scatter {3,4};
#                     odd  i: DMA {0,1}   + scatter {2,3,4}.
# The first N_LP pairs use the light scatter for BOTH rows (their odd-di2
# slab is DMA'd too, bdma slot 5) so the Pool engine builds phase margin
# before the heavy scatters start.
SC_E = (3, 4)        # scattered di, even i (and odd i of light pairs)
SC_O = (2, 3, 4)     # scattered di, odd i of heavy pairs
NSL_E = 2 * len(SC_E) * K * 2   # 40 scatter slots per partition, light
NSL_O = 2 * len(SC_O) * K * 2   # 60, heavy
N_LP = 0             # light pairs
N_WARM = 9           # PE warmup matmuls (cover startup DMA latency)
N_WARM_SM = 2        # small trailing warmups (fine-grained alignment)


def md_off(p):
    """maskd column offset of pair p (light pairs hold 80 slots, heavy 100)."""
    return 2 * NSL_E * p if p < N_LP else (
        2 * NSL_E * N_LP + (NSL_E + NSL_O) * (p - N_LP))


def _mi(x):
    return getattr(x, "ins", x)


def relocate_sync(pres, scats, posts):
    """Move the scatters' semaphore waits onto `pres` and updates onto
    `posts` (all chained in Pool-engine program order via nosync deps; Q7
    execution is strict FIFO per engine, so advancing waits and delaying
    updates across the group is sync-preserving).  Waits merge by max per
    semaphore, updates merge by sum."""
    def si_of(inst):
        si = inst.sync_info
        if si is None:
            return [], []
        return list(si.on_wait or []), list(si.on_update or [])

    wmax, uacc = {}, {}
    for s in scats:
        w, u = si_of(_mi(s))
        for x in w:
            assert x.sync_type == "semaphore" and x.wait_mode == "sem-ge-imm", x
            prev = wmax.get(x.id)
            if prev is None or x.wait_value > prev.wait_value:
                wmax[x.id] = x
        for x in u:
            assert x.sync_type == "semaphore" and x.update_mode in (
                "sem-inc", "sem-add-imm"), x
            prev = uacc.get(x.id)
            if prev is None:
                uacc[x.id] = mybir.SyncUpdate(
                    sync_type="semaphore", id=x.id, ant_name=x.ant_name,
                    update_mode="sem-add-imm", update_value=x.update_value)
            else:
                prev.update_value = prev.update_value + x.update_value
        _mi(s).sync_info = mybir.SyncInfo(on_wait=[], on_update=[])

    for carrier in pres:
        ci = _mi(carrier)
        cw, cu = si_of(ci)
        for w in cw:
            inc = wmax.pop(w.id, None)
            if inc is not None and inc.wait_value > w.wait_value:
                w.wait_value = inc.wait_value
        take = list(wmax.values())
        wmax.clear()
        ci.sync_info = mybir.SyncInfo(on_wait=cw + take, on_update=cu)
        break
    assert not wmax

    for carrier in posts:
        ci = _mi(carrier)
        cw, cu = si_of(ci)
        for u in cu:
            inc = uacc.pop(u.id, None)
            if inc is not None:
                u.update_value = u.update_value + inc.update_value
                u.update_mode = "sem-add-imm"
        take = list(uacc.values())
        uacc.clear()
        ci.sync_info = mybir.SyncInfo(on_wait=cw, on_update=cu + take)
        break
    assert not uacc


def split_sync(nc):
    """Enforce <=1 wait and <=1 update per instruction (this walrus build's
    events capacity), hoisting excess waits onto standalone same-engine
    sequencer NOPs placed immediately before (sync-equivalent).  Also hoists
    a wait that shares its semaphore with the instruction's own update."""
    for f in nc.m.functions:
        for b in f.blocks:
            lst = b.instructions
            i = 0
            while i < len(lst):
                inst = lst[i]
                si = getattr(inst, "sync_info", None)
                if si is None:
                    i += 1
                    continue
                w = list(si.on_wait or [])
                u = list(si.on_update or [])
                assert len(u) <= 1, (inst.name, u)
                uids = {x.id for x in u}
                conflict = any(x.id in uids for x in w) or (
                    w and any(x.update_mode == "sem-add-imm" for x in u))
                if len(w) <= 1 and not conflict:
                    i += 1
                    continue
                if (w and w[-1].id not in uids
                        and not any(x.update_mode == "sem-add-imm" for x in u)):
                    move, keep = w[:-1], w[-1:]
                else:
                    move, keep = w, []
                for wt in move:
                    nop = mybir.InstNoOp(
                        name=f"{inst.name}-ss{i}", text_hint="syncsplit")
                    nop.engine = inst.engine
                    nop.sync_info = mybir.SyncInfo(on_wait=[wt], on_update=[])
                    nc.register_instruction(nop, overwrite=True)
                    lst.insert(i, nop)
                    i += 1
                inst.sync_info = mybir.SyncInfo(on_wait=keep, on_update=u)
                i += 1


def _slots(mask_shard, i, scs):
    """Scatter payload for row-pair i over the di set `scs`:
    [j', (yp, dr, dj, px)] = m[(scs[dr], dj), 2i+yp, 2j'-2dj+4+px]
    (0 when the target column is OOB; those slots have idx -1)."""
    kk, ny, sw = mask_shard.shape
    mv = mask_shard.reshape(K, K, ny // 2, 2, sw)  # [di, dj, i, yp, x]
    d = np.zeros((128, 2, len(scs), K, 2), dtype=np.float16)
    for dj in range(K):
        for px in range(2):
            x = 2 * np.arange(128) - 2 * dj + 4 + px  # [128]
            valid = (x >= 0) & (x < sw)
            xc = np.clip(x, 0, sw - 1)
            for dr, di in enumerate(scs):
                sel = mv[di, dj, i][:, xc] * valid[None, :]  # [yp, 128]
                d[:, :, dr, dj, px] = sel.T
    return d.reshape(128, -1)


def host_maskd(mask_shard: np.ndarray):
    """Concatenated scatter payloads: per pair, even-i slots then odd-i
    slots ({3,4} for light pairs, {2,3,4} for heavy)."""
    ni = mask_shard.shape[1] // 2
    parts = []
    for p in range(ni // 2):
        parts.append(_slots(mask_shard, 2 * p, SC_E))
        parts.append(_slots(mask_shard, 2 * p + 1,
                            SC_E if p < N_LP else SC_O))
    return np.ascontiguousarray(np.concatenate(parts, axis=1))


def host_bidx():
    """Static scatter index tables into the [2*ns*SW u16] dst regions
    (layout [yp, dr, x]); -1 marks OOB slots (ignored by the scatter).
    Concatenated [even (40) | odd (60)] -> [128, 100]."""
    def table(scs):
        ns = len(scs)
        idx = np.full((128, 2, ns, K, 2), -1, dtype=np.int16)
        for j in range(128):
            for yp in range(2):
                for dr in range(ns):
                    for dj in range(K):
                        for px in range(2):
                            x = 2 * j - 2 * dj + 4 + px
                            if 0 <= x < SW:
                                idx[j, yp, dr, dj, px] = (yp * ns + dr) * SW + x
        return idx.reshape(128, -1)

    return np.ascontiguousarray(
        np.concatenate([table(SC_E), table(SC_O)], axis=1))


def host_bdma(mask_shard: np.ndarray):
    """Dense banded slabs per i-pair: bd[j', p, yp, slot, x] with
    slots 0..2 = even-i di {0,1,2}, slots 3..4 = odd-i di {0,1},
    slot 5 = odd-i di 2 (filled for all pairs, DMA'd only for light ones):
    value = m[(di, dj), y, x] at dj = j'-x//2+2, zero outside the band."""
    kk, ny, sw = mask_shard.shape
    ni = ny // 2
    mv = mask_shard.reshape(K, K, ni, 2, sw)  # [di, dj, i, yp, x]
    bd = np.zeros((128, ni // 2, 2, 6, sw), dtype=np.float16)
    for dj in range(K):
        for px in range(2):
            x = 2 * np.arange(128) - 2 * dj + 4 + px  # [128] per j'
            valid = (x >= 0) & (x < sw)
            jv = np.arange(128)[valid]
            xv = x[valid]
            for slot, (par, di) in enumerate(
                    [(0, 0), (0, 1), (0, 2), (1, 0), (1, 1), (1, 2)]):
                # [ni/2, yp, nv] -> [nv, ni/2, yp]
                bd[jv, :, :, slot, xv] = (
                    mv[di, dj][par::2][:, :, xv].transpose(2, 0, 1))
    return np.ascontiguousarray(bd)


def build_program(n_i: int = N_I, r_in: int = R_IN, relocate: bool = True,
                  detect_races: bool = False):
    nc = bass.Bass(detect_race_conditions=detect_races)

    md_total = md_off(n_i // 2)
    featt = nc.dram_tensor("featt", [128, r_in, C], F16, kind="ExternalInput")
    # flat [j', pair-slots] so DMA descriptors stay >=512B contiguous
    maskd = nc.dram_tensor("maskd", [128, md_total], F16,
                           kind="ExternalInput")
    bidx = nc.dram_tensor(
        "bidx", [128, NSL_E + NSL_O], I16, kind="ExternalInput")
    bdma = nc.dram_tensor(
        "bdma", [128, n_i // 2, 2, 6, SW], F16, kind="ExternalInput")
    out = nc.dram_tensor("out", [C, 2 * n_i, SW], F16, kind="ExternalOutput")

    groups = []

    with tile.TileContext(nc) as tc:
        with (
            tc.tile_pool(name="const", bufs=1) as constp,
            tc.tile_pool(name="ft", bufs=1) as ftp,
            tc.tile_pool(name="maskd", bufs=1) as mdp,
            tc.tile_pool(name="bc", bufs=7) as bcp,
            tc.tile_pool(name="btile", bufs=4) as bp,
            tc.tile_pool(name="orow", bufs=4) as orowp,
            tc.tile_pool(name="mm", bufs=6, space="PSUM") as mmp,
            tc.tile_pool(name="mmwarm", bufs=1, space="PSUM") as mmwp,
        ):
            nc.gpsimd.load_library(library_config.local_scatter)
            # PE warmup: keep the tensor engine continuously busy from t~0
            # so the p-state ramp completes while the startup DMAs land and
            # every real matmul runs at full clock.  Reads uninitialized
            # SBUF; the warm PSUM bank is never read back.
            warm = constp.tile([128, 512 + 128], F16, tag="warm")
            warm_pm = mmwp.tile([128, 512], F32, tag="mmwarm")
            nc.gpsimd.memset(warm[:], 0.0)
            for _ in range(N_WARM):
                nc.tensor.matmul(
                    warm_pm[:], warm[:, 512:640], warm[:, 0:512],
                    start=True, stop=True)
            for _ in range(N_WARM_SM):
                nc.tensor.matmul(
                    warm_pm[:, 0:128], warm[:, 512:640], warm[:, 0:128],
                    start=True, stop=True)

            # All DMAs issue on SP so their service order follows program
            # order: startup loads critical-path first (scatter table +
            # payload for early i's, feature rows 0-5, banded slabs for
            # i 0-1), then the rest interleaved.
            bix = constp.tile([128, NSL_E + NSL_O], I16, tag="bix")
            nc.sync.dma_start(out=bix[:], in_=bidx[:])
            # sync-carrier anchor for the relocated scatter semaphores
            panchor = constp.tile([128, 4], F16, tag="panchor")

            md = mdp.tile([128, md_total], F16)
            md3 = md_off(3)
            nc.sync.dma_start(out=md[:, :md3], in_=maskd[:, :md3])

            ft = ftp.tile([128, r_in, C], F16)
            bcs = {}

            def load_bc(p):
                bct = bcp.tile([128, 2, 6, SW], F16, tag="bc", name="bc")
                bcs[p] = bct
                nsl = 6 if p < N_LP else 5
                nc.sync.dma_start(
                    out=bct[:, :, 0:nsl, :], in_=bdma[:, p, :, 0:nsl])

            nc.sync.dma_start(out=ft[:, 0:5, :], in_=featt[:, 0:5, :])
            # first pair's slabs split by parity so i=0 can start sooner
            bc0 = bcp.tile([128, 2, 6, SW], F16, tag="bc", name="bc")
            bcs[0] = bc0
            nc.sync.dma_start(out=bc0[:, :, 0:3, :], in_=bdma[:, 0, :, 0:3])
            nc.sync.dma_start(out=ft[:, 5:12, :], in_=featt[:, 5:12, :])
            nc.sync.dma_start(out=bc0[:, :, 3:5, :], in_=bdma[:, 0, :, 3:5])
            load_bc(1)

            # remaining loads are issued inside the loop, positioned by
            # when their consumer runs, so early DMA serves only what is
            # imminent (SP program order == DMA service order).
            md9 = md_off(10)
            deferred = {
                0: [lambda: load_bc(2)],
                2: [lambda: nc.sync.dma_start(
                    out=ft[:, 12:18, :], in_=featt[:, 12:18, :])],
                4: [lambda: nc.sync.dma_start(
                    out=ft[:, 18:24, :], in_=featt[:, 18:24, :])],
                6: [lambda: nc.sync.dma_start(
                    out=ft[:, 24:30, :], in_=featt[:, 24:30, :])],
                8: [lambda: nc.sync.dma_start(
                    out=ft[:, 30:r_in, :], in_=featt[:, 30:r_in, :])],
                1: [lambda: nc.sync.dma_start(
                    out=md[:, md3:md9], in_=maskd[:, md3:md9])],
                7: [lambda: nc.sync.dma_start(
                    out=md[:, md9:], in_=maskd[:, md9:])],
            }
            for oi in range(1, 26, 2):
                deferred.setdefault(oi, []).append(
                    lambda pp=(oi + 5) // 2: load_bc(pp))

            # ---- main loop over output row pairs ----
            for i in range(n_i):
                par = i % 2
                p = i // 2
                for fn in deferred.get(i, ()):
                    fn()
                # scatter region [yp, dr, x]
                heavy = par == 1 and p >= N_LP
                ns = len(SC_O) if heavy else len(SC_E)
                bt = bp.tile([128, 2, ns, SW], F16,
                             tag=f"bt{int(heavy)}", name="bt")
                pre = nc.gpsimd.memset(panchor[:], 0.0)
                if groups:
                    groups[-1][2] = pre  # pre also carries prev updates
                    _add_dep(_mi(pre), _mi(groups[-1][1][-1]), sync=False,
                             reason="chain")
                moff = md_off(p) + (0 if par == 0 else NSL_E)
                nsl = NSL_O if heavy else NSL_E
                sc = nc.gpsimd.local_scatter(
                    out_ap=bt[:].bitcast(U16),
                    data_ap=md[:, moff:moff + nsl].bitcast(U16),
                    idxs_ap=bix[:, NSL_E:NSL_E + NSL_O] if heavy
                    else bix[:, 0:NSL_E],
                    channels=128,
                    num_elems=2 * ns * SW,
                    num_idxs=nsl,
                )
                _add_dep(_mi(sc), _mi(pre), sync=False, reason="chain")
                groups.append([pre, [sc], None])

                orow = orowp.tile([128, 2, 2, SW], F16, tag="orow")
                bcv = bcs[p]  # [128, 2, 6, SW]: slots e0 e1 e2 o0 o1 o2
                nd = 2 if heavy else 3
                slot0 = 0 if par == 0 else 3
                # last row pair: ch0 last, so the final output goes through
                # the single-engine ACT copy+DMA path (shortest tail)
                for ch in ((1, 0) if i == n_i - 1 else (0, 1)):
                    pm = mmp.tile([128, 2, SW], F32, tag="mm")
                    # operands: DMA'd slabs (di < nd) and scattered slabs;
                    # for i=0 the scatter lands first, so emit its matmuls
                    # first (start/stop flags follow emission order)
                    ops = [(di, bcv[:, :, slot0 + di, :])
                           for di in range(nd)]
                    ops += [(nd + dr, bt[:, :, dr, :]) for dr in range(ns)]
                    if i == 0:
                        ops = ops[nd:] + ops[:nd]
                    for k, (di, rhs) in enumerate(ops):
                        nc.tensor.matmul(
                            pm[:],
                            ft[:, i + di, ch * 128:(ch + 1) * 128],
                            rhs,
                            start=(k == 0),
                            stop=(k == K - 1),
                        )
                    if ch == 0:
                        nc.scalar.copy(
                            out=orow[:, :, ch, :], in_=pm[:])
                    else:
                        nc.vector.tensor_copy(
                            orow[:, :, ch, :], pm[:])
                # per-i output DMAs, split across the two HWDGE issue
                # engines (ch0 from ACT right after its copy, ch1 from SP)
                # so their sequencer costs parallelize; the last pair issues
                # both from ACT to keep the tail on one fast path
                for ch, eng in ((0, nc.scalar), (1, nc.sync)):
                    eng.dma_start(
                        out=out[ch * 128:(ch + 1) * 128,
                                2 * i:2 * i + 2, :],
                        in_=orow[:, :, ch, :],
                    )
            term = nc.gpsimd.memset(panchor[:], 0.0)
            _add_dep(_mi(term), _mi(groups[-1][1][-1]), sync=False,
                     reason="chain")
            groups[-1][2] = term

    if relocate:
        for pre, scats, post in groups:
            relocate_sync([pre], scats, [post])
        split_sync(nc)
    return nc


def finalize_for_hw(nc):
    assert mybir.codegen_inst_isa_subclasses(nc)
    return nc


_PROGRAM = None


def _get_program():
    global _PROGRAM
    if _PROGRAM is None:
        _PROGRAM = finalize_for_hw(build_program())
    return _PROGRAM


def kernel(features: np.ndarray, masks: np.ndarray) -> np.ndarray:
    from concourse.bass_utils import run_bass_kernel_spmd

    features = np.ascontiguousarray(features, dtype=np.float32)
    masks = np.ascontiguousarray(masks, dtype=np.float32)
    fpad = np.pad(features, ((0, 0), (0, 0), (PAD, PAD), (0, 0)))
    bix = host_bidx()

    in_maps = []
    for core in range(N_CORES):
        n, q = divmod(core, 4)
        ftt = fpad[n, :, QH * q:QH * q + R_IN, :].transpose(2, 1, 0)
        mshard = masks[n, :, 2 * N_I * q:2 * N_I * (q + 1), :]
        in_maps.append({
            "featt": np.ascontiguousarray(ftt, dtype=np.float16),
            "maskd": host_maskd(mshard),
            "bidx": bix,
            "bdma": host_bdma(mshard),
        })

    nc = _get_program()
    trace = os.environ.get("CARAFE_TRACE") == "1"
    res = run_bass_kernel_spmd(nc, in_maps, list(range(N_CORES)), trace=trace)
    kernel.last_results = res

    out = np.empty((N, C, SH, SW), dtype=np.float32)
    for core in range(N_CORES):
        n, q = divmod(core, 4)
        out[n, :, 2 * N_I * q:2 * N_I * (q + 1), :] = (
            res.results[core]["out"].astype(np.float32))
    return out
